# revision 3
# baseline (speedup 1.0000x reference)
"""CommNet Trainium2 kernel.

Reference computation (per batch element b, n=128 agents, hid=256):
    h = tanh(obs @ enc_W + enc_b)
    iter 0:   h = tanh(h @ f_W + f_b + C_b)                       (c = 0)
    iter 1-3: c = (sum_i h_i - h) / (n-1)
              h = tanh(h @ f_W + c @ C_W + f_b + C_b)
    action = log_softmax(h @ act_W + act_b); value = h @ val_W + val_b

Sharding: data-parallel over batch, 16/8 = 2 batch elements per core.

On-chip layout: h is kept transposed (hid on partitions as 2 k-tiles of 128,
b*n = 256 on the free axis) so the comm iterations are pure matmuls with no
transposes.  Two algebraic rewrites:
    h@f_W + c@C_W = h@(f_W - C_W/127) + S@(C_W/127),  S[b] = sum_i h[b,i]
and S comes for free from the accum_out (free-axis sum) of the previous
iteration's tanh activation, so no reduction ops are on the critical path.
"""

import numpy as np

_B, _N, _OBS, _HID, _ACT = 16, 128, 128, 256, 32
_NCORES = 8
_BPC = _B // _NCORES  # batch elements per core
_P = 128
_KT = _HID // _P  # hid k-tiles
_ROWS = _BPC * _N  # rows (= free axis length) per core
_INV = 1.0 / (_N - 1.0)

_CACHE = {}


def _build_nc(debug=False):
    import concourse.bacc as bacc
    import concourse.mybir as mybir
    import concourse.tile as tile
    from concourse.masks import make_identity

    dt = mybir.dt.float32
    AF = mybir.ActivationFunctionType
    AX = mybir.AxisListType

    nc = bacc.Bacc("TRN2", target_bir_lowering=False, debug=debug)

    obs = nc.dram_tensor("obs", [_BPC, _N, _OBS], dt, kind="ExternalInput")
    enc_W = nc.dram_tensor("enc_W", [_OBS, _HID], dt, kind="ExternalInput")
    enc_b = nc.dram_tensor("enc_b", [_HID, 1], dt, kind="ExternalInput")
    f_W = nc.dram_tensor("f_W", [_HID, _HID], dt, kind="ExternalInput")
    f_b = nc.dram_tensor("f_b", [_HID, 1], dt, kind="ExternalInput")
    C_W = nc.dram_tensor("C_W", [_HID, _HID], dt, kind="ExternalInput")
    C_b = nc.dram_tensor("C_b", [_HID, 1], dt, kind="ExternalInput")
    act_W = nc.dram_tensor("act_W", [_HID, _ACT], dt, kind="ExternalInput")
    act_b = nc.dram_tensor("act_b", [_ACT, 1], dt, kind="ExternalInput")
    val_W = nc.dram_tensor("val_W", [_HID, 1], dt, kind="ExternalInput")
    val_b = nc.dram_tensor("val_b", [1, 1], dt, kind="ExternalInput")
    action = nc.dram_tensor("action", [_ROWS, _ACT], dt, kind="ExternalOutput")
    value = nc.dram_tensor("value", [1, _ROWS], dt, kind="ExternalOutput")

    with tile.TileContext(nc) as tc:
        with (
            tc.tile_pool(name="w", bufs=1) as W,
            tc.tile_pool(name="h", bufs=6) as H,
            tc.tile_pool(name="sm", bufs=4) as SM,
            tc.tile_pool(name="ps", bufs=3, space="PSUM") as PS,
            tc.tile_pool(name="pT", bufs=2, space="PSUM") as PT,
            tc.tile_pool(name="ptr", bufs=2, space="PSUM") as PTR,
        ):
            # ---- weight / bias loads (all fp32, natural layouts) ----
            encW_s = W.tile([_P, _HID], dt, tag="encW")
            nc.sync.dma_start(out=encW_s, in_=enc_W[:, :])
            fW_s, CW_s, actW_s, valW_s = [], [], [], []
            for k in range(_KT):
                t = W.tile([_P, _HID], dt, tag=f"fW{k}")
                nc.sync.dma_start(out=t, in_=f_W[k * _P:(k + 1) * _P, :])
                fW_s.append(t)
                t = W.tile([_P, _HID], dt, tag=f"CW{k}")
                nc.sync.dma_start(out=t, in_=C_W[k * _P:(k + 1) * _P, :])
                CW_s.append(t)
                t = W.tile([_P, _ACT], dt, tag=f"aW{k}")
                nc.sync.dma_start(out=t, in_=act_W[k * _P:(k + 1) * _P, :])
                actW_s.append(t)
                t = W.tile([_P, 1], dt, tag=f"vW{k}")
                nc.sync.dma_start(out=t, in_=val_W[k * _P:(k + 1) * _P, :])
                valW_s.append(t)
            encb_s, fb_s, Cb_s = [], [], []
            for m in range(_KT):
                t = W.tile([_P, 1], dt, tag=f"encb{m}")
                nc.sync.dma_start(out=t, in_=enc_b[m * _P:(m + 1) * _P, :])
                encb_s.append(t)
                t = W.tile([_P, 1], dt, tag=f"fb{m}")
                nc.sync.dma_start(out=t, in_=f_b[m * _P:(m + 1) * _P, :])
                fb_s.append(t)
                t = W.tile([_P, 1], dt, tag=f"Cb{m}")
                nc.sync.dma_start(out=t, in_=C_b[m * _P:(m + 1) * _P, :])
                Cb_s.append(t)
            actb_s = W.tile([_ACT, 1], dt, tag="actb")
            nc.sync.dma_start(out=actb_s, in_=act_b[:, :])
            valb_s = W.tile([1, 1], dt, tag="valb")
            nc.sync.dma_start(out=valb_s, in_=val_b[:, :])

            ident = W.tile([_P, _P], dt, tag="ident")
            make_identity(nc, ident)

            # ---- derived weights: Cs = C_W/127, G = f_W - Cs, bfc = f_b+C_b ----
            Cs, G, bfc = [], [], []
            for k in range(_KT):
                cs = W.tile([_P, _HID], dt, tag=f"Cs{k}")
                nc.vector.tensor_scalar_mul(cs, CW_s[k], _INV)
                Cs.append(cs)
                g = W.tile([_P, _HID], dt, tag=f"G{k}")
                nc.vector.tensor_sub(g, fW_s[k], cs)
                G.append(g)
            for m in range(_KT):
                t = W.tile([_P, 1], dt, tag=f"bfc{m}")
                nc.vector.tensor_add(t, fb_s[m], Cb_s[m])
                bfc.append(t)

            # ---- obs load + PE transpose to [obs, rows] ----
            obsT = W.tile([_P, _ROWS], dt, tag="obsT")
            for b in range(_BPC):
                ob = SM.tile([_P, _OBS], dt, tag="obsload")
                nc.sync.dma_start(out=ob, in_=obs[b])
                pt = PTR.tile([_P, _P], dt, tag="tr")
                nc.tensor.transpose(pt, ob, ident)
                nc.vector.tensor_copy(out=obsT[:, b * _N:(b + 1) * _N], in_=pt)

            # ---- encoder: hT[m] = tanh(enc_W[:,m]^T @ obsT + enc_b[m]) ----
            hT = []
            for m in range(_KT):
                ps = PS.tile([_P, _ROWS], dt, tag="ps")
                nc.tensor.matmul(ps, encW_s[:, m * _P:(m + 1) * _P], obsT,
                                 start=True, stop=True)
                ht = H.tile([_P, _ROWS], dt, tag="hT")
                nc.scalar.activation(out=ht, in_=ps, func=AF.Tanh,
                                     bias=encb_s[m][:, 0:1])
                hT.append(ht)

            # ---- comm iter 0: h = tanh(h @ f_W + f_b + C_b); collect S ----
            hT2, S = [], []
            for m in range(_KT):
                ps = PS.tile([_P, _ROWS], dt, tag="ps")
                for k in range(_KT):
                    nc.tensor.matmul(ps, fW_s[k][:, m * _P:(m + 1) * _P], hT[k],
                                     start=(k == 0), stop=(k == _KT - 1))
                ht = H.tile([_P, _ROWS], dt, tag="hT")
                s = SM.tile([_P, _BPC], dt, tag="S")
                for b in range(_BPC):
                    nc.scalar.activation(out=ht[:, b * _N:(b + 1) * _N],
                                         in_=ps[:, b * _N:(b + 1) * _N],
                                         func=AF.Tanh, bias=bfc[m][:, 0:1],
                                         accum_out=s[:, b:b + 1])
                hT2.append(ht)
                S.append(s)
            hT = hT2

            # ---- comm iters 1..3 ----
            for it in range(1, 4):
                last = it == 3
                Tb = []
                for m in range(_KT):
                    pt = PT.tile([_P, _BPC], dt, tag="pT")
                    for k in range(_KT):
                        nc.tensor.matmul(pt, Cs[k][:, m * _P:(m + 1) * _P], S[k],
                                         start=(k == 0), stop=(k == _KT - 1))
                    tb = SM.tile([_P, _BPC], dt, tag="Tb")
                    nc.vector.tensor_scalar_add(tb, pt, bfc[m][:, 0:1])
                    Tb.append(tb)
                hT2, S2 = [], []
                for m in range(_KT):
                    ps = PS.tile([_P, _ROWS], dt, tag="ps")
                    for k in range(_KT):
                        nc.tensor.matmul(ps, G[k][:, m * _P:(m + 1) * _P], hT[k],
                                         start=(k == 0), stop=(k == _KT - 1))
                    ht = H.tile([_P, _ROWS], dt, tag="hT")
                    s = None if last else SM.tile([_P, _BPC], dt, tag="S")
                    for b in range(_BPC):
                        nc.scalar.activation(
                            out=ht[:, b * _N:(b + 1) * _N],
                            in_=ps[:, b * _N:(b + 1) * _N],
                            func=AF.Tanh, bias=Tb[m][:, b:b + 1],
                            accum_out=None if s is None else s[:, b:b + 1])
                    hT2.append(ht)
                    if s is not None:
                        S2.append(s)
                hT = hT2
                S = S2

            # ---- value head: v^T = val_W^T @ hT + val_b  ([1, rows]) ----
            psv = PT.tile([1, _ROWS], dt, tag="pT")
            for k in range(_KT):
                nc.tensor.matmul(psv, valW_s[k], hT[k],
                                 start=(k == 0), stop=(k == _KT - 1))
            vsb = SM.tile([1, _ROWS], dt, tag="vsb")
            nc.scalar.activation(out=vsb, in_=psv, func=AF.Identity,
                                 bias=valb_s[0:1, 0:1])
            nc.sync.dma_start(out=value[:, :], in_=vsb)

            # ---- action head: logits^T = act_W^T @ hT + act_b ([32, rows]) ----
            psa = PS.tile([_ACT, _ROWS], dt, tag="ps")
            for k in range(_KT):
                nc.tensor.matmul(psa, actW_s[k], hT[k],
                                 start=(k == 0), stop=(k == _KT - 1))
            asb = SM.tile([_ACT, _ROWS], dt, tag="asb")
            nc.scalar.activation(out=asb, in_=psa, func=AF.Identity,
                                 bias=actb_s[:, 0:1])
            # per batch element: transpose back to [n, act], log-softmax rows
            for b in range(_BPC):
                pat = PTR.tile([_P, _ACT], dt, tag="tr")
                nc.tensor.transpose(pat, asb[:, b * _N:(b + 1) * _N],
                                    ident[0:_ACT, 0:_ACT])
                negmx = SM.tile([_P, 1], dt, tag="negmx")
                nc.vector.reduce_max(out=negmx, in_=pat, axis=AX.X, negate=True)
                ex = SM.tile([_P, _ACT], dt, tag="ex")
                se = SM.tile([_P, 1], dt, tag="se")
                nc.scalar.activation(out=ex, in_=pat, func=AF.Exp,
                                     bias=negmx[:, 0:1], accum_out=se[:, 0:1])
                lse = SM.tile([_P, 1], dt, tag="lse")
                nc.scalar.activation(out=lse, in_=se, func=AF.Ln)
                ofs = SM.tile([_P, 1], dt, tag="ofs")
                nc.vector.tensor_sub(ofs, negmx, lse)
                aout = SM.tile([_P, _ACT], dt, tag="aout")
                nc.vector.tensor_scalar_add(aout, pat, ofs[:, 0:1])
                nc.sync.dma_start(out=action[b * _N:(b + 1) * _N, :], in_=aout)

    return nc


def _in_maps(inputs):
    obs = np.ascontiguousarray(np.asarray(inputs["obs"], dtype=np.float32))
    wk = {}
    for name in ("enc_W", "f_W", "C_W", "act_W", "val_W"):
        wk[name] = np.ascontiguousarray(np.asarray(inputs[name], np.float32))
    for name, dim in (("enc_b", _HID), ("f_b", _HID), ("C_b", _HID),
                      ("act_b", _ACT), ("val_b", 1)):
        wk[name] = np.ascontiguousarray(
            np.asarray(inputs[name], np.float32).reshape(dim, 1))
    maps = []
    for c in range(_NCORES):
        m = dict(wk)
        m["obs"] = np.ascontiguousarray(obs[c * _BPC:(c + 1) * _BPC])
        maps.append(m)
    return maps


def _run(inputs, trace=False):
    from concourse.bass_utils import run_bass_kernel_spmd

    if "nc" not in _CACHE:
        nc = _build_nc(debug=False)
        nc.compile()
        _CACHE["nc"] = nc
    nc = _CACHE["nc"]
    try:
        res = run_bass_kernel_spmd(nc, _in_maps(inputs), list(range(_NCORES)),
                                   trace=trace)
    except ModuleNotFoundError:
        # axon NTFF profiling hook unavailable in this container
        res = run_bass_kernel_spmd(nc, _in_maps(inputs), list(range(_NCORES)),
                                   trace=False)
    action = np.concatenate(
        [res.results[c]["action"].reshape(_BPC, _N, _ACT)
         for c in range(_NCORES)], axis=0)
    value = np.concatenate(
        [res.results[c]["value"].reshape(_BPC, _N, 1)
         for c in range(_NCORES)], axis=0)
    return (action, value), res


def kernel(**inputs):
    (action, value), _ = _run(inputs)
    return action, value
